# revision 8
# baseline (speedup 1.0000x reference)
"""Trainium2 Bass kernel for GQA attention block (dense_transformer).

Full-input contract: kernel(**inputs) takes the unsharded tensors from
setup_inputs() and returns the full [2, 2048, 2048] output.

Sharding: 8 cores = 2 (batch) x 4 (head groups). Each core computes
attention for 8 Q heads / 2 KV heads of one batch element plus its
partial output projection; the host sums the 4 head-group partials.

Self-contained: shapes hardcoded for B=2, S=2048, D=2048, 32 Q/8 KV
heads, head_dim 64.
"""
import numpy as np
from contextlib import ExitStack

import concourse.bass as bass
import concourse.tile as tile
import concourse.mybir as mybir
from concourse import bacc
from concourse.masks import make_identity
from concourse.bass_utils import run_bass_kernel_spmd

F32 = mybir.dt.float32
F32R = mybir.dt.float32r
AF = mybir.ActivationFunctionType
OP = mybir.AluOpType

B, S, D = 2, 2048, 2048
N_HEAD, N_KV_HEAD = 32, 8
HD = 64
NH, NKV = 8, 2           # per-core Q heads / KV heads
EQ = NH * HD             # 512 local q dim
EKV = NKV * HD           # 128 local k (or v) dim
SC = S // 128            # 16 s-chunks of 128
IC = S // 512            # 4 i-chunks of 512
KO = D // 128            # 16 contraction chunks

_NC_CACHE = {}


def build_nc():
    if "nc" in _NC_CACHE:
        return _NC_CACHE["nc"]
    nc = bacc.Bacc()
    xT = nc.declare_dram_parameter("xT", [D, S], F32R, isOutput=False)
    wqkvT = nc.declare_dram_parameter("wqkvT", [D, EQ + 2 * EKV], F32R, isOutput=False)
    woT = nc.declare_dram_parameter("woT", [EQ, D], F32R, isOutput=False)
    f0 = nc.declare_dram_parameter("f0", [S, HD // 2], F32, isOutput=False)
    f1 = nc.declare_dram_parameter("f1", [S, HD // 2], F32, isOutput=False)
    masks = nc.declare_dram_parameter("masks", [4, 128, 512], F32, isOutput=False)
    y = nc.declare_dram_parameter("y", [S, D], F32, isOutput=True)

    with tile.TileContext(nc) as tc:
        with ExitStack() as store_ab:
            # stores that live phase A -> B
            stq = store_ab.enter_context(tc.tile_pool(name="stq", bufs=1))
            qTp = [stq.tile([128, S], F32R, tag=f"qTp{j}", name=f"qTp{j}") for j in range(NH // 2)]
            kTp = stq.tile([128, S], F32R, tag="kTp", name="kTp")
            # V per s-chunk: [v_h0(64) | 1 | v_h1(64) | 1]
            vst = stq.tile([128, SC, 2 * (HD + 1)], F32R, tag="vst")

            # ---------------- Phase A: projection + rope + transpose --------
            with ExitStack() as pa:
                wpool = pa.enter_context(tc.tile_pool(name="wpool", bufs=1))
                xpool = pa.enter_context(tc.tile_pool(name="xpool", bufs=2))
                fpool = pa.enter_context(tc.tile_pool(name="fpool", bufs=1))
                rpool = pa.enter_context(tc.tile_pool(name="rpool", bufs=3))
                psa = pa.enter_context(tc.tile_pool(name="psa", bufs=6, space="PSUM"))

                wq = wpool.tile([128, KO, EQ + 2 * EKV], F32R)
                for ko in range(KO):
                    nc.sync.dma_start(wq[:, ko], wqkvT[ko * 128:(ko + 1) * 128, :])
                f0t = fpool.tile([128, SC, HD // 2], F32)
                f1t = fpool.tile([128, SC, HD // 2], F32)
                nc.sync.dma_start(f0t[:], f0.rearrange("(sc p) i -> p sc i", p=128))
                nc.sync.dma_start(f1t[:], f1.rearrange("(sc p) i -> p sc i", p=128))
                ident = fpool.tile([128, 128], F32)
                make_identity(nc, ident[:])

                for ic in range(IC):
                    xslab = xpool.tile([128, KO, 512], F32R, tag="xslab")
                    for ko in range(KO):
                        nc.sync.dma_start(
                            xslab[:, ko], xT[ko * 128:(ko + 1) * 128, ic * 512:(ic + 1) * 512]
                        )
                    for sc2 in range(4):
                        sc = ic * 4 + sc2
                        psQ = psa.tile([128, EQ], F32, tag="ps")
                        psKV = psa.tile([128, 2 * EKV], F32, tag="ps")
                        for ko in range(KO):
                            lhsT = xslab[:, ko, sc2 * 128:(sc2 + 1) * 128]
                            nc.tensor.matmul(
                                psQ[:], lhsT, wq[:, ko, 0:EQ],
                                start=(ko == 0), stop=(ko == KO - 1),
                            )
                            nc.tensor.matmul(
                                psKV[:], lhsT, wq[:, ko, EQ:],
                                start=(ko == 0), stop=(ko == KO - 1),
                            )
                        # ---- RoPE on Q ----
                        rotq = rpool.tile([128, EQ], F32R, tag="rotq")
                        ta = rpool.tile([128, NH, 32], F32, tag="ta")
                        tb = rpool.tile([128, NH, 32], F32, tag="tb")
                        q3 = psQ[:].rearrange("p (h z i) -> p h z i", h=NH, z=2)
                        r3 = rotq[:].rearrange("p (h z i) -> p h z i", h=NH, z=2)
                        f0b = f0t[:, sc, None, :].to_broadcast([128, NH, 32])
                        f1b = f1t[:, sc, None, :].to_broadcast([128, NH, 32])
                        nc.vector.tensor_tensor(ta[:], q3[:, :, 0], f0b, OP.mult)
                        nc.vector.tensor_tensor(tb[:], q3[:, :, 1], f1b, OP.mult)
                        nc.vector.tensor_tensor(r3[:, :, 0], ta[:], tb[:], OP.subtract)
                        nc.vector.tensor_tensor(ta[:], q3[:, :, 1], f0b, OP.mult)
                        nc.vector.tensor_tensor(tb[:], q3[:, :, 0], f1b, OP.mult)
                        nc.vector.tensor_tensor(r3[:, :, 1], ta[:], tb[:], OP.add)
                        # ---- RoPE on K ----
                        rotk = rpool.tile([128, EKV], F32R, tag="rotk")
                        k3 = psKV[:, 0:EKV].rearrange("p (h z i) -> p h z i", h=NKV, z=2)
                        rk3 = rotk[:].rearrange("p (h z i) -> p h z i", h=NKV, z=2)
                        f0k = f0t[:, sc, None, :].to_broadcast([128, NKV, 32])
                        f1k = f1t[:, sc, None, :].to_broadcast([128, NKV, 32])
                        tc_ = rpool.tile([128, NKV, 32], F32, tag="tc")
                        td = rpool.tile([128, NKV, 32], F32, tag="td")
                        nc.vector.tensor_tensor(tc_[:], k3[:, :, 0], f0k, OP.mult)
                        nc.vector.tensor_tensor(td[:], k3[:, :, 1], f1k, OP.mult)
                        nc.vector.tensor_tensor(rk3[:, :, 0], tc_[:], td[:], OP.subtract)
                        nc.vector.tensor_tensor(tc_[:], k3[:, :, 1], f0k, OP.mult)
                        nc.vector.tensor_tensor(td[:], k3[:, :, 0], f1k, OP.mult)
                        nc.vector.tensor_tensor(rk3[:, :, 1], tc_[:], td[:], OP.add)
                        # ---- V copyback (+ ones columns) ----
                        nc.vector.tensor_copy(
                            vst[:, sc, 0:HD], psKV[:, EKV:EKV + HD]
                        )
                        nc.vector.tensor_copy(
                            vst[:, sc, HD + 1:2 * HD + 1],
                            psKV[:, EKV + HD:],
                        )
                        nc.vector.memset(vst[:, sc, HD:HD + 1].bitcast(F32), 1.0)
                        nc.vector.memset(vst[:, sc, 2 * HD + 1:].bitcast(F32), 1.0)
                        # ---- transposes: natural [s, e] -> [e, s] stores ----
                        for ec in range(4):
                            pst = psa.tile([128, 128], F32, tag="ps")
                            nc.tensor.transpose(
                                pst[:], rotq[:, ec * 128:(ec + 1) * 128].bitcast(F32),
                                ident[:],
                            )
                            for half in range(2):
                                h_ = 2 * ec + half
                                nc.vector.tensor_copy(
                                    qTp[h_ % 4][(h_ // 4) * 64:(h_ // 4) * 64 + 64,
                                                sc * 128:(sc + 1) * 128],
                                    pst[half * 64:half * 64 + 64, :],
                                )
                        pst = psa.tile([128, 128], F32, tag="ps")
                        nc.tensor.transpose(pst[:], rotk[:].bitcast(F32), ident[:])
                        nc.vector.tensor_copy(
                            kTp[:, sc * 128:(sc + 1) * 128], pst[:]
                        )

            # ---------------- Phase B: attention ---------------------------
            with ExitStack() as sbc:
                sto = sbc.enter_context(tc.tile_pool(name="sto", bufs=1))
                oT = sto.tile([128, 4, S], F32R)  # attn-out^T, d-chunks x s

                with ExitStack() as pb:
                    cpool = pb.enter_context(tc.tile_pool(name="cpool", bufs=1))
                    ppool = pb.enter_context(tc.tile_pool(name="ppool", bufs=4))
                    spool = pb.enter_context(tc.tile_pool(name="spool", bufs=3))
                    psb = pb.enter_context(tc.tile_pool(name="psb", bufs=6, space="PSUM"))

                    mk = cpool.tile([128, 4, 512], F32)
                    for d in range(4):
                        nc.sync.dma_start(mk[:, d], masks[d])
                    ones_t = cpool.tile([1, 64], F32R)
                    nc.vector.memset(ones_t[:].bitcast(F32), 1.0)

                    for h in range(NH):
                        kv = h // 4
                        for ic in range(IC):
                            njb = 4 * (ic + 1)
                            psO = psb.tile([65, 512], F32, tag="ps")
                            for jb in range(njb):
                                psS = psb.tile([128, 512], F32, tag="ps")
                                nc.tensor.matmul(
                                    psS[:],
                                    kTp[kv * 64:kv * 64 + 64, jb * 128:(jb + 1) * 128],
                                    qTp[h % 4][(h // 4) * 64:(h // 4) * 64 + 64,
                                               ic * 512:(ic + 1) * 512],
                                    start=True, stop=True,
                                )
                                pt = ppool.tile([128, 512], F32R, tag="pt")
                                nc.scalar.activation(pt[:], psS[:], AF.Exp, scale=0.125)
                                dblk = jb - 4 * ic
                                if dblk >= 0:
                                    nc.vector.tensor_tensor(
                                        pt[:], pt[:].bitcast(F32),
                                        mk[:, dblk], OP.mult,
                                    )
                                nc.tensor.matmul(
                                    psO[:],
                                    vst[:, jb, kv * (HD + 1):(kv + 1) * (HD + 1)],
                                    pt[:],
                                    start=(jb == 0), stop=(jb == njb - 1),
                                )
                            rs = spool.tile([1, 512], F32R, tag="rs")
                            with nc.allow_low_precision(reason="f32r storage of reciprocal"):
                                nc.vector.reciprocal(rs[:], psO[64:65, :])
                            psBc = psb.tile([64, 512], F32, tag="ps")
                            nc.tensor.matmul(psBc[:], ones_t[:], rs[:], start=True, stop=True)
                            bb = spool.tile([64, 512], F32, tag="bb")
                            nc.scalar.copy(bb[:], psBc[:])
                            nc.vector.tensor_tensor(
                                oT[(h % 2) * 64:(h % 2) * 64 + 64, h // 2,
                                   ic * 512:(ic + 1) * 512],
                                psO[0:64, :], bb[:], OP.mult,
                            )

                # ---------------- Phase C: output projection ----------------
                with ExitStack() as pc:
                    wopool = pc.enter_context(tc.tile_pool(name="wopool", bufs=1))
                    ypool = pc.enter_context(tc.tile_pool(name="ypool", bufs=3))
                    psc = pc.enter_context(tc.tile_pool(name="psc", bufs=4, space="PSUM"))

                    wo_t = wopool.tile([128, 4, D], F32R)
                    for dc in range(4):
                        nc.sync.dma_start(wo_t[:, dc], woT[dc * 128:(dc + 1) * 128, :])

                    for sc in range(SC):
                        for ec4 in range(4):
                            psY = psc.tile([128, 512], F32, tag="ps")
                            for dc in range(4):
                                nc.tensor.matmul(
                                    psY[:],
                                    oT[:, dc, sc * 128:(sc + 1) * 128],
                                    wo_t[:, dc, ec4 * 512:(ec4 + 1) * 512],
                                    start=(dc == 0), stop=(dc == 3),
                                )
                            yt = ypool.tile([128, 512], F32, tag="yt")
                            nc.vector.tensor_copy(yt[:], psY[:])
                            nc.sync.dma_start(
                                y[sc * 128:(sc + 1) * 128, ec4 * 512:(ec4 + 1) * 512],
                                yt[:],
                            )

    nc.compile()
    _NC_CACHE["nc"] = nc
    return nc


def _pair_split_perm(n_heads):
    """Row permutation putting even dims then odd dims within each head."""
    idx = []
    for h in range(n_heads):
        base = h * HD
        idx.extend([base + 2 * i for i in range(HD // 2)])
        idx.extend([base + 2 * i + 1 for i in range(HD // 2)])
    return np.array(idx)


def make_in_maps(x, freqs_cis, wqkv, wo):
    x = np.asarray(x, dtype=np.float32)
    freqs_cis = np.asarray(freqs_cis, dtype=np.float32)
    wqkv = np.asarray(wqkv, dtype=np.float32)
    wo = np.asarray(wo, dtype=np.float32)

    f0 = np.ascontiguousarray(freqs_cis[:, :, 0])
    f1 = np.ascontiguousarray(freqs_cis[:, :, 1])
    jj = np.arange(128)[:, None]
    ii = np.arange(512)[None, :]
    masks = np.stack(
        [(jj + d * 128 <= ii).astype(np.float32) for d in range(4)], axis=0
    )
    qperm = _pair_split_perm(NH)
    kperm = _pair_split_perm(NKV)

    xT = [np.ascontiguousarray(x[b].T) for b in range(B)]
    in_maps = []
    for c in range(8):
        b, g = c // 4, c % 4
        wq_g = wqkv[g * EQ:(g + 1) * EQ][qperm]              # [512, D]
        wk_g = wqkv[D + g * EKV:D + (g + 1) * EKV][kperm]    # [128, D]
        wv_g = wqkv[D + N_KV_HEAD * HD + g * EKV:
                    D + N_KV_HEAD * HD + (g + 1) * EKV]      # [128, D]
        wqkvT_g = np.ascontiguousarray(
            np.concatenate([wq_g, wk_g, wv_g], axis=0).T
        )                                                     # [D, 768]
        woT_g = np.ascontiguousarray(wo[:, g * EQ:(g + 1) * EQ].T)  # [512, D]
        in_maps.append({
            "xT": xT[b],
            "wqkvT": wqkvT_g,
            "woT": woT_g,
            "f0": f0,
            "f1": f1,
            "masks": masks,
        })
    return in_maps


def kernel(x, freqs_cis, wqkv, wo, trace=False):
    nc = build_nc()
    in_maps = make_in_maps(x, freqs_cis, wqkv, wo)
    res = run_bass_kernel_spmd(nc, in_maps, core_ids=list(range(8)), trace=trace)
    outs = [np.asarray(r["y"]) for r in res.results]
    out = np.empty((B, S, D), dtype=np.float32)
    for b in range(B):
        out[b] = outs[4 * b] + outs[4 * b + 1] + outs[4 * b + 2] + outs[4 * b + 3]
    if trace:
        return out, res
    return out
